# revision 15
# baseline (speedup 1.0000x reference)
"""Single-head attention on 8 TRN2 NeuronCores (Bass/Tile).

Problem: x[4, 4096, 1024] f32; Wq/Wk/Wv[1024, 64]; bq/bk/bv[64];
out = softmax((x@Wq)(x@Wk)^T / 8) @ (x@Wv)  -> [4, 4096, 64] f32.

Sharding: core i handles batch b=i//2, sequence half h=i%2 (2048 query
rows). K/V are computed on each half locally and pair-wise AllGathered
(sequence-parallel attention across 2 devices per batch). Attention runs
in two passes — local k-half first (overlapping the AllGather), then the
remote k-half, whose shard is fetched from the collective output with a
partition-id-derived dynamic offset (peer rank = (pid+1)%2).

All transposes are regular identity matmuls (cheaper than PE transpose
mode and they keep the PE HAM-warm): out = in.T @ I.

Per-core dataflow (matmuls bf16, f32 PSUM accumulation):
  x[2048,1024] --HWDGE--> SBUF f32 --DVE cast--> bf16 --PE @I-->
    xT[1024, 2048] (evictions batched 8-blocks-per-2-banks, ACT copies)
  KT/VT/QT[64, t] = W^T x^T   (PSUM accum over 8 d-chunks, DVE evict+bias)
  AllGather(KT_loc, VT_loc) within pairs (runs under pass 1)
  V[k, 65] = (VT@I)^T with ones column appended (row-sum trick)
  per pass, per q-tile of 512:
    for k-group of 2x128:  S^T[k,q] = KT_chunk^T @ QT  (2 PSUM banks)
    exp via ACT (scale=1/8) PSUM->SBUF bf16 = P^T
    O^T[65, q] += Vplus_chunk^T @ P^T  (PSUM accum; row 64 = denom)
  pass1 evicts O^T to SBUF; pass2 adds its PSUM into it, then per q-tile
  transposes O^T back, normalizes by recip(col 64) and stores.
"""

import numpy as np

import concourse.bass as bass
import concourse.mybir as mybir
import concourse.tile as tile
from concourse import bacc
from concourse.bass_utils import run_bass_kernel_spmd
from concourse.masks import make_identity

B, T, D, H = 4, 4096, 1024, 64
N_CORES = 8
TL = T // 2          # 2048 local rows per core
DCH = D // 128       # 8 contraction chunks
NQT = TL // 512      # 4 q tiles of 512
NKC_H = TL // 128    # 16 k chunks of 128 per half
KGRP = 2             # k-chunks per exp batch (2 PSUM banks)

F32 = mybir.dt.float32
BF16 = mybir.dt.bfloat16
AF = mybir.ActivationFunctionType

GROUPS = [(g, KGRP) for g in range(0, NKC_H, KGRP)]


def build_nc():
    nc = bacc.Bacc("TRN2", target_bir_lowering=False, debug=False,
                   num_devices=N_CORES)
    x = nc.declare_dram_parameter("x", [TL, D], F32, isOutput=False)
    wq = nc.declare_dram_parameter("wq", [D, H], F32, isOutput=False)
    bq = nc.declare_dram_parameter("bq", [H], F32, isOutput=False)
    wk = nc.declare_dram_parameter("wk", [D, H], F32, isOutput=False)
    bk = nc.declare_dram_parameter("bk", [H], F32, isOutput=False)
    wv = nc.declare_dram_parameter("wv", [D, H], F32, isOutput=False)
    bv = nc.declare_dram_parameter("bv", [H], F32, isOutput=False)
    out = nc.declare_dram_parameter("out", [TL, H], F32, isOutput=True)

    with tile.TileContext(nc) as tc:
        with (
            tc.tile_pool(name="const", bufs=1) as const,
            tc.tile_pool(name="big", bufs=1) as big,
            tc.tile_pool(name="xload", bufs=6) as xload,
            tc.tile_pool(name="onat", bufs=3) as onat_pool,
            tc.tile_pool(name="pt", bufs=4) as pt_pool,
            tc.tile_pool(name="dram", bufs=1, space="DRAM") as dram,
        ):
            # ---- constants (identities first: transposes need them;
            #      W/bias DMAs are issued after the x loads below so the
            #      small transfers don't delay the big ones) ----
            ident_bf = const.tile([128, 128], BF16, tag="ident_bf")
            make_identity(nc, ident_bf[:])
            ident_f32 = const.tile([128, 128], F32, tag="ident_f32")
            make_identity(nc, ident_f32[:])

            w_sb = {}
            b_sb = {}
            # weights + biases ride the SWDGE queue so they don't delay the
            # big x loads on the two HWDGE rings
            for name, wd, bd in (("q", wq, bq), ("k", wk, bk), ("v", wv, bv)):
                wt = const.tile([128, DCH, H], BF16, tag=f"w{name}")
                # cast f32->bf16 in the (SWDGE) DMA; W row d = c*128 + p
                nc.gpsimd.dma_start(
                    out=wt[:], in_=wd.rearrange("(c p) h -> p c h", p=128))
                w_sb[name] = wt
                bt = const.tile([H, 1], F32, tag=f"b{name}")
                nc.gpsimd.dma_start(
                    out=bt[:], in_=bd.rearrange("(h o) -> h o", o=1))
                b_sb[name] = bt

            # ---- big persistent SBUF tensors ----
            xT = big.tile([128, DCH, TL], BF16, tag="xT")
            qT = big.tile([H, TL], BF16, tag="qT")
            kT_loc = big.tile([H, TL], BF16, tag="kT_loc")
            vT_loc = big.tile([H, TL], BF16, tag="vT_loc")
            kT_rem = big.tile([H, TL], BF16, tag="kT_rem")
            vT_rem = big.tile([H, TL], BF16, tag="vT_rem")
            # vplus chunks: 0..15 local half, 16..31 remote half
            vplus = big.tile([128, 2 * NKC_H, H + 1], BF16, tag="vplus")
            oT = big.tile([H + 1, TL], F32, tag="oT")

            kv_in = dram.tile([2, H, TL], BF16)
            kv_out_k = dram.tile([2, H, TL], BF16)
            kv_out_v = dram.tile([2, H, TL], BF16)
            nc.vector.memset(vplus[:, :, H:H + 1], 1.0)

            def transpose_v_half(vt_src, half, pool):
                # (VT chunk).T via identity matmul, 8 chunks per PSUM bank
                for g in range(2):
                    pv = pool.tile([128, 8, H], F32, tag="pv")
                    for j in range(8):
                        kc = g * 8 + j
                        nc.tensor.matmul(
                            pv[:, j, :],
                            lhsT=vt_src[:, kc * 128:(kc + 1) * 128],
                            rhs=ident_bf[:H, :H], start=True, stop=True)
                    nc.vector.tensor_copy(
                        out=vplus[:, half * NKC_H + g * 8:
                                  half * NKC_H + (g + 1) * 8, :H],
                        in_=pv[:])

            # ---- phase A: x^T, projections, AllGather kick ----
            with (
                tc.tile_pool(name="ps_tx", bufs=2, space="PSUM") as ps_tx,
                tc.tile_pool(name="ps_proj", bufs=2, space="PSUM") as ps_proj,
                tc.tile_pool(name="ps_pva", bufs=2, space="PSUM") as ps_pva,
            ):
                # x load (HWDGE) + DVE cast + identity-matmul transpose.
                # 8 transposed blocks fill one [128, 8, 128] f32 PSUM tile
                # (2 banks); one batched ACT copy evicts + casts to bf16.
                for tt in range(TL // 128):
                    # SWDGE (lane 2) carries the tail tiles: its queue first
                    # delivers the weights/biases above
                    lane = 2 if tt >= 11 else tt % 2
                    xb = xload.tile([128, D], BF16, tag="xb")
                    if lane == 2:
                        # SWDGE casts f32->bf16 in the DMA datapath
                        nc.gpsimd.dma_start(
                            out=xb[:], in_=x[tt * 128:(tt + 1) * 128, :])
                    else:
                        eng = nc.sync if lane == 0 else nc.scalar
                        xf = xload.tile([128, D], F32, tag="xf")
                        eng.dma_start(
                            out=xf[:], in_=x[tt * 128:(tt + 1) * 128, :])
                        nc.vector.tensor_copy(out=xb[:], in_=xf[:])
                    ptx = ps_tx.tile([128, DCH, 128], F32, tag="ptx")
                    for c in range(DCH):
                        nc.tensor.matmul(
                            ptx[:, c, :], lhsT=xb[:, c * 128:(c + 1) * 128],
                            rhs=ident_bf[:], start=True, stop=True)
                    nc.scalar.activation(
                        out=xT[:, :, tt * 128:(tt + 1) * 128], in_=ptx[:],
                        func=AF.Copy)

                def project(dst, w, b):
                    for qt in range(NQT):
                        ps = ps_proj.tile([H, 512], F32, tag="proj")
                        for c in range(DCH):
                            nc.tensor.matmul(
                                ps[:], lhsT=w[:, c, :],
                                rhs=xT[:, c, qt * 512:(qt + 1) * 512],
                                start=(c == 0), stop=(c == DCH - 1))
                        nc.vector.tensor_scalar_add(
                            dst[:, qt * 512:(qt + 1) * 512], ps[:], b[:])

                rg = [[0, 1], [2, 3], [4, 5], [6, 7]]
                project(kT_loc, w_sb["k"], b_sb["k"])
                nc.sync.dma_start(out=kv_in[0], in_=kT_loc[:])
                nc.gpsimd.collective_compute(
                    "AllGather", mybir.AluOpType.bypass, replica_groups=rg,
                    ins=[kv_in[0].opt()], outs=[kv_out_k.opt()])
                # Q next: it gates the first pass-1 S-matmul. V's AllGather
                # queues behind K's on the collective engine anyway.
                project(qT, w_sb["q"], b_sb["q"])
                project(vT_loc, w_sb["v"], b_sb["v"])
                nc.sync.dma_start(out=kv_in[1], in_=vT_loc[:])
                nc.gpsimd.collective_compute(
                    "AllGather", mybir.AluOpType.bypass, replica_groups=rg,
                    ins=[kv_in[1].opt()], outs=[kv_out_v.opt()])

                transpose_v_half(vT_loc, 0, ps_pva)

            def attn_pass(kt_src, base, qt, po):
                qs = qT[:, qt * 512:(qt + 1) * 512]
                for (g0, gn) in GROUPS:
                    ps = ps_s.tile([128, KGRP * 512], F32, tag="ps")
                    for j in range(gn):
                        nc.tensor.matmul(
                            ps[:, j * 512:(j + 1) * 512],
                            lhsT=kt_src[:, (g0 + j) * 128:(g0 + j + 1) * 128],
                            rhs=qs, start=True, stop=True)
                    ptile = pt_pool.tile([128, KGRP * 512], BF16, tag="pt")
                    nc.scalar.activation(
                        out=ptile[:, :gn * 512], in_=ps[:, :gn * 512],
                        func=AF.Exp, scale=1.0 / np.sqrt(H))
                    for j in range(gn):
                        kc = g0 + j
                        nc.tensor.matmul(
                            po[:], lhsT=vplus[:, base + kc, :],
                            rhs=ptile[:, j * 512:(j + 1) * 512],
                            start=(kc == 0), stop=(kc == NKC_H - 1))

            # ---- phase B: two attention passes (pass 1 overlaps the AG),
            #      with the output epilogue interleaved into pass 2 ----
            with (
                tc.tile_pool(name="ps_s", bufs=2, space="PSUM") as ps_s,
                tc.tile_pool(name="ps_o", bufs=1, space="PSUM") as ps_o,
                tc.tile_pool(name="ps_pvb", bufs=1, space="PSUM") as ps_pvb,
                tc.tile_pool(name="ps_ot", bufs=2, space="PSUM") as ps_ot,
            ):
                # pass 1: local k-half
                for qt in range(NQT):
                    po = ps_o.tile([H + 1, 512], F32, tag="po")
                    attn_pass(kT_loc, 0, qt, po)
                    nc.vector.tensor_copy(
                        out=oT[:, qt * 512:(qt + 1) * 512], in_=po[:])

                # remote shard fetch: peer rank within the pair
                rm = (nc.partition_id() + 1) % 2
                nc.sync.dma_start(out=kT_rem[:], in_=kv_out_k[bass.ds(rm, 1)])
                nc.sync.dma_start(out=vT_rem[:], in_=kv_out_v[bass.ds(rm, 1)])
                transpose_v_half(vT_rem, 1, ps_pvb)

                # pass 2: remote k-half, accumulated into oT + epilogue
                for qt in range(NQT):
                    po = ps_o.tile([H + 1, 512], F32, tag="po")
                    attn_pass(kT_rem, NKC_H, qt, po)
                    nc.vector.tensor_add(
                        oT[:, qt * 512:(qt + 1) * 512],
                        oT[:, qt * 512:(qt + 1) * 512], po[:])
                    # epilogue: transpose back, normalize, store
                    for sub in range(4):
                        st = qt * 4 + sub
                        pot = ps_ot.tile([128, H + 1], F32, tag="pot")
                        nc.tensor.matmul(
                            pot[:], lhsT=oT[:, st * 128:(st + 1) * 128],
                            rhs=ident_f32[:H + 1, :H + 1],
                            start=True, stop=True)
                        rs = onat_pool.tile([128, 1], F32, tag="rs")
                        nc.vector.reciprocal(rs[:], pot[:, H:H + 1])
                        on = onat_pool.tile([128, H], F32, tag="on")
                        nc.vector.tensor_scalar_mul(on[:], pot[:, :H], rs[:])
                        nc.sync.dma_start(
                            out=out[st * 128:(st + 1) * 128, :], in_=on[:])

    nc.finalize()
    return nc


_NC_CACHE = None


def _get_nc():
    global _NC_CACHE
    if _NC_CACHE is None:
        _NC_CACHE = build_nc()
    return _NC_CACHE


def _make_shards(inputs):
    x = np.asarray(inputs["x"], dtype=np.float32)
    shards = []
    for i in range(N_CORES):
        b, h = divmod(i, 2)
        shards.append({
            "x": np.ascontiguousarray(x[b, h * TL:(h + 1) * TL, :]),
            "wq": np.asarray(inputs["Wq"], dtype=np.float32),
            "bq": np.asarray(inputs["bq"], dtype=np.float32),
            "wk": np.asarray(inputs["Wk"], dtype=np.float32),
            "bk": np.asarray(inputs["bk"], dtype=np.float32),
            "wv": np.asarray(inputs["Wv"], dtype=np.float32),
            "bv": np.asarray(inputs["bv"], dtype=np.float32),
        })
    return shards


def kernel(**inputs):
    nc = _get_nc()
    res = run_bass_kernel_spmd(nc, _make_shards(inputs), list(range(N_CORES)))
    out = np.empty((B, T, H), dtype=np.float32)
    for i in range(N_CORES):
        b, h = divmod(i, 2)
        out[b, h * TL:(h + 1) * TL, :] = res.results[i]["out"]
    return out


# revision 17
# speedup vs baseline: 1.0268x; 1.0268x over previous
"""Single-head attention on 8 TRN2 NeuronCores (Bass/Tile).

Problem: x[4, 4096, 1024] f32; Wq/Wk/Wv[1024, 64]; bq/bk/bv[64];
out = softmax((x@Wq)(x@Wk)^T / 8) @ (x@Wv)  -> [4, 4096, 64] f32.

Sharding: core i handles batch b=i//2, sequence half h=i%2 (2048 query
rows). K/V are computed on each half locally and pair-wise AllGathered
(sequence-parallel attention across 2 devices per batch). Attention runs
in two passes — local k-half first (overlapping the AllGather), then the
remote k-half, whose shard is fetched from the collective output with a
partition-id-derived dynamic offset (peer rank = (pid+1)%2).

All transposes are regular identity matmuls (cheaper than PE transpose
mode and they keep the PE HAM-warm): out = in.T @ I.

Per-core dataflow (matmuls bf16, f32 PSUM accumulation):
  x[2048,1024] --HWDGE--> SBUF f32 --DVE cast--> bf16 --PE @I-->
    xT[1024, 2048] (evictions batched 8-blocks-per-2-banks, ACT copies)
  KT/VT/QT[64, t] = W^T x^T   (PSUM accum over 8 d-chunks, DVE evict+bias)
  AllGather(KT_loc, VT_loc) within pairs (runs under pass 1)
  V[k, 65] = (VT@I)^T with ones column appended (row-sum trick)
  per pass, per q-tile of 512:
    for k-group of 2x128:  S^T[k,q] = KT_chunk^T @ QT  (2 PSUM banks)
    exp via ACT (scale=1/8) PSUM->SBUF bf16 = P^T
    O^T[65, q] += Vplus_chunk^T @ P^T  (PSUM accum; row 64 = denom)
  pass1 evicts O^T to SBUF; pass2 adds its PSUM into it, then per q-tile
  transposes O^T back, normalizes by recip(col 64) and stores.
"""

import numpy as np

import concourse.bass as bass
import concourse.mybir as mybir
import concourse.tile as tile
from concourse import bacc
from concourse.bass_utils import run_bass_kernel_spmd
from concourse.masks import make_identity

B, T, D, H = 4, 4096, 1024, 64
N_CORES = 8
TL = T // 2          # 2048 local rows per core
DCH = D // 128       # 8 contraction chunks
NQT = TL // 512      # 4 q tiles of 512
NKC_H = TL // 128    # 16 k chunks of 128 per half
KGRP = 2             # k-chunks per exp batch (2 PSUM banks)

F32 = mybir.dt.float32
BF16 = mybir.dt.bfloat16
AF = mybir.ActivationFunctionType

GROUPS = [(g, KGRP) for g in range(0, NKC_H, KGRP)]


def build_nc():
    nc = bacc.Bacc("TRN2", target_bir_lowering=False, debug=False,
                   num_devices=N_CORES)
    x = nc.declare_dram_parameter("x", [TL, D], F32, isOutput=False)
    wq = nc.declare_dram_parameter("wq", [D, H], F32, isOutput=False)
    bq = nc.declare_dram_parameter("bq", [H], F32, isOutput=False)
    wk = nc.declare_dram_parameter("wk", [D, H], F32, isOutput=False)
    bk = nc.declare_dram_parameter("bk", [H], F32, isOutput=False)
    wv = nc.declare_dram_parameter("wv", [D, H], F32, isOutput=False)
    bv = nc.declare_dram_parameter("bv", [H], F32, isOutput=False)
    out = nc.declare_dram_parameter("out", [TL, H], F32, isOutput=True)

    with tile.TileContext(nc) as tc:
        with (
            tc.tile_pool(name="const", bufs=1) as const,
            tc.tile_pool(name="big", bufs=1) as big,
            tc.tile_pool(name="xload", bufs=6) as xload,
            tc.tile_pool(name="xbig", bufs=2) as xbig,
            tc.tile_pool(name="onat", bufs=3) as onat_pool,
            tc.tile_pool(name="pt", bufs=4) as pt_pool,
            tc.tile_pool(name="dram", bufs=1, space="DRAM") as dram,
        ):
            # ---- constants (identities first: transposes need them;
            #      W/bias DMAs are issued after the x loads below so the
            #      small transfers don't delay the big ones) ----
            ident_bf = const.tile([128, 128], BF16, tag="ident_bf")
            make_identity(nc, ident_bf[:])
            ident_f32 = const.tile([128, 128], F32, tag="ident_f32")
            make_identity(nc, ident_f32[:])

            w_sb = {}
            b_sb = {}
            # weights + biases ride the SWDGE queue so they don't delay the
            # big x loads on the two HWDGE rings
            for name, wd, bd in (("q", wq, bq), ("k", wk, bk), ("v", wv, bv)):
                wt = const.tile([128, DCH, H], BF16, tag=f"w{name}")
                # cast f32->bf16 in the (SWDGE) DMA; W row d = c*128 + p
                nc.gpsimd.dma_start(
                    out=wt[:], in_=wd.rearrange("(c p) h -> p c h", p=128))
                w_sb[name] = wt
                bt = const.tile([H, 1], F32, tag=f"b{name}")
                nc.gpsimd.dma_start(
                    out=bt[:], in_=bd.rearrange("(h o) -> h o", o=1))
                b_sb[name] = bt

            # ---- big persistent SBUF tensors ----
            xT = big.tile([128, DCH, TL], BF16, tag="xT")
            qT = big.tile([H, TL], BF16, tag="qT")
            kT_loc = big.tile([H, TL], BF16, tag="kT_loc")
            vT_loc = big.tile([H, TL], BF16, tag="vT_loc")
            kT_rem = big.tile([H, TL], BF16, tag="kT_rem")
            vT_rem = big.tile([H, TL], BF16, tag="vT_rem")
            # vplus chunks: 0..15 local half, 16..31 remote half
            vplus = big.tile([128, 2 * NKC_H, H + 1], BF16, tag="vplus")
            oT = big.tile([H + 1, TL], F32, tag="oT")

            kv_in = dram.tile([2, H, TL], BF16)
            kv_out_k = dram.tile([2, H, TL], BF16)
            kv_out_v = dram.tile([2, H, TL], BF16)
            nc.vector.memset(vplus[:, :, H:H + 1], 1.0)

            def transpose_v_half(vt_src, half, pool):
                # (VT chunk).T via identity matmul, 8 chunks per PSUM bank
                for g in range(2):
                    pv = pool.tile([128, 8, H], F32, tag="pv")
                    for j in range(8):
                        kc = g * 8 + j
                        nc.tensor.matmul(
                            pv[:, j, :],
                            lhsT=vt_src[:, kc * 128:(kc + 1) * 128],
                            rhs=ident_bf[:H, :H], start=True, stop=True)
                    nc.vector.tensor_copy(
                        out=vplus[:, half * NKC_H + g * 8:
                                  half * NKC_H + (g + 1) * 8, :H],
                        in_=pv[:])

            # ---- phase A: x^T, projections, AllGather kick ----
            with (
                tc.tile_pool(name="ps_tx", bufs=2, space="PSUM") as ps_tx,
                tc.tile_pool(name="ps_proj", bufs=2, space="PSUM") as ps_proj,
                tc.tile_pool(name="ps_pva", bufs=2, space="PSUM") as ps_pva,
            ):
                # x load (HWDGE, both rings) + DVE cast + identity-matmul
                # transpose. First 4 tiles load individually (fast start),
                # the rest in 2 MB DMAs (better ring throughput). 8
                # transposed blocks fill one [128, 8, 128] f32 PSUM tile
                # (2 banks); one batched ACT copy evicts + casts to bf16.
                def xpipe_tile(xb_tile, tt):
                    nc.vector.tensor_copy(
                        out=xb_tile[:], in_=xf_of[tt][0][:, xf_of[tt][1], :])
                    ptx = ps_tx.tile([128, DCH, 128], F32, tag="ptx")
                    for c in range(DCH):
                        nc.tensor.matmul(
                            ptx[:, c, :],
                            lhsT=xb_tile[:, c * 128:(c + 1) * 128],
                            rhs=ident_bf[:], start=True, stop=True)
                    nc.scalar.activation(
                        out=xT[:, :, tt * 128:(tt + 1) * 128], in_=ptx[:],
                        func=AF.Copy)

                xf_of = {}
                for tt in range(4):
                    eng = nc.sync if tt % 2 == 0 else nc.scalar
                    xf = xload.tile([128, 1, D], F32, tag="xf")
                    eng.dma_start(
                        out=xf[:], in_=x[tt * 128:(tt + 1) * 128, :].rearrange(
                            "(j p) d -> p j d", p=128))
                    xf_of[tt] = (xf, 0)
                for g in range(3):
                    eng = [nc.sync, nc.scalar, nc.sync][g]
                    xf = xbig.tile([128, 4, D], F32, tag="xf4")
                    r0 = 512 + g * 512
                    eng.dma_start(
                        out=xf[:], in_=x[r0:r0 + 512, :].rearrange(
                            "(j p) d -> p j d", p=128))
                    for j in range(4):
                        xf_of[4 + g * 4 + j] = (xf, j)
                for tt in range(TL // 128):
                    xb = xload.tile([128, D], BF16, tag="xb")
                    xpipe_tile(xb, tt)

                def project(dst, w, b):
                    for qt in range(NQT):
                        ps = ps_proj.tile([H, 512], F32, tag="proj")
                        for c in range(DCH):
                            nc.tensor.matmul(
                                ps[:], lhsT=w[:, c, :],
                                rhs=xT[:, c, qt * 512:(qt + 1) * 512],
                                start=(c == 0), stop=(c == DCH - 1))
                        nc.vector.tensor_scalar_add(
                            dst[:, qt * 512:(qt + 1) * 512], ps[:], b[:])

                rg = [[0, 1], [2, 3], [4, 5], [6, 7]]
                project(kT_loc, w_sb["k"], b_sb["k"])
                nc.sync.dma_start(out=kv_in[0], in_=kT_loc[:])
                nc.gpsimd.collective_compute(
                    "AllGather", mybir.AluOpType.bypass, replica_groups=rg,
                    ins=[kv_in[0].opt()], outs=[kv_out_k.opt()])
                # Q next: it gates the first pass-1 S-matmul. V's AllGather
                # queues behind K's on the collective engine anyway.
                project(qT, w_sb["q"], b_sb["q"])
                project(vT_loc, w_sb["v"], b_sb["v"])
                nc.sync.dma_start(out=kv_in[1], in_=vT_loc[:])
                nc.gpsimd.collective_compute(
                    "AllGather", mybir.AluOpType.bypass, replica_groups=rg,
                    ins=[kv_in[1].opt()], outs=[kv_out_v.opt()])

                transpose_v_half(vT_loc, 0, ps_pva)

            def attn_pass(kt_src, base, qt, po):
                qs = qT[:, qt * 512:(qt + 1) * 512]
                for (g0, gn) in GROUPS:
                    ps = ps_s.tile([128, KGRP * 512], F32, tag="ps")
                    for j in range(gn):
                        nc.tensor.matmul(
                            ps[:, j * 512:(j + 1) * 512],
                            lhsT=kt_src[:, (g0 + j) * 128:(g0 + j + 1) * 128],
                            rhs=qs, start=True, stop=True)
                    ptile = pt_pool.tile([128, KGRP * 512], BF16, tag="pt")
                    nc.scalar.activation(
                        out=ptile[:, :gn * 512], in_=ps[:, :gn * 512],
                        func=AF.Exp, scale=1.0 / np.sqrt(H))
                    for j in range(gn):
                        kc = g0 + j
                        nc.tensor.matmul(
                            po[:], lhsT=vplus[:, base + kc, :],
                            rhs=ptile[:, j * 512:(j + 1) * 512],
                            start=(kc == 0), stop=(kc == NKC_H - 1))

            # ---- phase B: two attention passes (pass 1 overlaps the AG),
            #      with the output epilogue interleaved into pass 2 ----
            with (
                tc.tile_pool(name="ps_s", bufs=2, space="PSUM") as ps_s,
                tc.tile_pool(name="ps_o", bufs=1, space="PSUM") as ps_o,
                tc.tile_pool(name="ps_pvb", bufs=1, space="PSUM") as ps_pvb,
                tc.tile_pool(name="ps_ot", bufs=2, space="PSUM") as ps_ot,
            ):
                # remote shard fetch (gated on the AllGathers, but issued
                # with high priority so the pass-1 -> pass-2 transition is
                # seamless): peer rank within the pair
                rm = (nc.partition_id() + 1) % 2
                nc.sync.dma_start(out=kT_rem[:], in_=kv_out_k[bass.ds(rm, 1)])
                nc.sync.dma_start(out=vT_rem[:], in_=kv_out_v[bass.ds(rm, 1)])
                transpose_v_half(vT_rem, 1, ps_pvb)

                # pass 1: local k-half
                for qt in range(NQT):
                    po = ps_o.tile([H + 1, 512], F32, tag="po")
                    attn_pass(kT_loc, 0, qt, po)
                    nc.vector.tensor_copy(
                        out=oT[:, qt * 512:(qt + 1) * 512], in_=po[:])

                # pass 2: remote k-half, accumulated into oT + epilogue
                for qt in range(NQT):
                    po = ps_o.tile([H + 1, 512], F32, tag="po")
                    attn_pass(kT_rem, NKC_H, qt, po)
                    nc.vector.tensor_add(
                        oT[:, qt * 512:(qt + 1) * 512],
                        oT[:, qt * 512:(qt + 1) * 512], po[:])
                    # epilogue: transpose back, normalize, store
                    for sub in range(4):
                        st = qt * 4 + sub
                        pot = ps_ot.tile([128, H + 1], F32, tag="pot")
                        nc.tensor.matmul(
                            pot[:], lhsT=oT[:, st * 128:(st + 1) * 128],
                            rhs=ident_f32[:H + 1, :H + 1],
                            start=True, stop=True)
                        rs = onat_pool.tile([128, 1], F32, tag="rs")
                        nc.vector.reciprocal(rs[:], pot[:, H:H + 1])
                        on = onat_pool.tile([128, H], F32, tag="on")
                        nc.vector.tensor_scalar_mul(on[:], pot[:, :H], rs[:])
                        nc.sync.dma_start(
                            out=out[st * 128:(st + 1) * 128, :], in_=on[:])

    nc.finalize()
    return nc


_NC_CACHE = None


def _get_nc():
    global _NC_CACHE
    if _NC_CACHE is None:
        _NC_CACHE = build_nc()
    return _NC_CACHE


def _make_shards(inputs):
    x = np.asarray(inputs["x"], dtype=np.float32)
    shards = []
    for i in range(N_CORES):
        b, h = divmod(i, 2)
        shards.append({
            "x": np.ascontiguousarray(x[b, h * TL:(h + 1) * TL, :]),
            "wq": np.asarray(inputs["Wq"], dtype=np.float32),
            "bq": np.asarray(inputs["bq"], dtype=np.float32),
            "wk": np.asarray(inputs["Wk"], dtype=np.float32),
            "bk": np.asarray(inputs["bk"], dtype=np.float32),
            "wv": np.asarray(inputs["Wv"], dtype=np.float32),
            "bv": np.asarray(inputs["bv"], dtype=np.float32),
        })
    return shards


def kernel(**inputs):
    nc = _get_nc()
    res = run_bass_kernel_spmd(nc, _make_shards(inputs), list(range(N_CORES)))
    out = np.empty((B, T, H), dtype=np.float32)
    for i in range(N_CORES):
        b, h = divmod(i, 2)
        out[b, h * TL:(h + 1) * TL, :] = res.results[i]["out"]
    return out
